# revision 3
# baseline (speedup 1.0000x reference)
"""Trainium2 Bass kernel for nn_LinearRecurrentCell.

Computes h_next = h_prev @ W_hh.T + x_projected_t with
B = 1048576 tokens, H = 128, f32.

Strategy (data-parallel over 8 NeuronCores, per the sharding hint):
- Shard the batch dim of x/h across the 8 cores (131072 tokens each),
  replicate the 64 KB W_hh.
- Per core the kernel is memory-bound: 192 MiB of HBM traffic vs tiny
  compute. We stream 1024-token macro-tiles ([128 part x 1024 free] f32,
  512 KiB per DMA, 4 KiB contiguous per partition).
- The contraction over H needs H on SBUF partitions, so each 128x128
  sub-tile of h is PE-transposed (to PSUM), copied back to SBUF by the
  scalar engine, matmul'd against W^T (PE), then the vector engine adds
  x and the result streams back out.
"""

import numpy as np

B = 1048576
H = 128
P = 128
N_CORES = 8
B_CORE = B // N_CORES  # 131072
TOK = 1024             # tokens per macro-tile
N_SUB = TOK // P       # 8 sub-tiles of 128 tokens
N_MACRO = B_CORE // TOK  # 128 macro-tiles per core

_CACHE: dict = {}


def _build_nc(b_core: int = B_CORE, num_devices: int = N_CORES):
    import concourse.mybir as mybir
    import concourse.tile as tile
    from concourse import bacc

    from concourse.masks import make_identity

    f32 = mybir.dt.float32
    n_macro = b_core // TOK

    nc = bacc.Bacc(
        "TRN2",
        target_bir_lowering=False,
        debug=False,
        enable_asserts=False,
        num_devices=num_devices,
    )

    x = nc.dram_tensor("x", [b_core, H], f32, kind="ExternalInput").ap()
    h = nc.dram_tensor("h", [b_core, H], f32, kind="ExternalInput").ap()
    w = nc.dram_tensor("w", [H, H], f32, kind="ExternalInput").ap()
    out = nc.dram_tensor("out", [b_core, H], f32, kind="ExternalOutput").ap()

    # Macro-tile views: token = m*TOK + p*N_SUB + n, so each partition's
    # slice of a macro-tile is N_SUB*H*4 = 4 KiB contiguous in DRAM.
    x_v = x.rearrange("(m p n) h -> m p (n h)", p=P, n=N_SUB)
    h_v = h.rearrange("(m p n) h -> m p (n h)", p=P, n=N_SUB)
    o_v = out.rearrange("(m p n) h -> m p (n h)", p=P, n=N_SUB)

    with tile.TileContext(nc) as tc:
        with (
            tc.tile_pool(name="consts", bufs=1) as consts,
            tc.tile_pool(name="sbuf", bufs=3) as sbuf,
            tc.tile_pool(name="tp_psum", bufs=2, space="PSUM") as tp_psum,
            tc.tile_pool(name="mm_psum", bufs=2, space="PSUM") as mm_psum,
        ):
            identity = consts.tile([P, P], f32)
            make_identity(nc, identity[:])

            # Build W^T in SBUF once: wt[h, g] = W[g, h].
            w_nat = consts.tile([P, P], f32)
            nc.sync.dma_start(out=w_nat[:], in_=w[:, :])
            wt_psum = tp_psum.tile([P, 512], f32, tag="tp")
            nc.tensor.transpose(wt_psum[:, :P], w_nat[:], identity[:])
            wt = consts.tile([P, P], f32)
            nc.vector.tensor_copy(out=wt[:], in_=wt_psum[:, :P])

            for m in range(n_macro):
                h_tile = sbuf.tile([P, TOK], f32, tag="h_tile")
                x_tile = sbuf.tile([P, TOK], f32, tag="x_tile")
                nc.sync.dma_start(out=h_tile[:], in_=h_v[m])
                nc.sync.dma_start(out=x_tile[:], in_=x_v[m])

                # Transpose the 8 [128,128] sub-tiles: 4 at a time into one
                # PSUM bank, then one scalar-engine copy back to SBUF.
                ht = sbuf.tile([P, TOK], f32, tag="ht")
                for half in range(TOK // 512):
                    pt = tp_psum.tile([P, 512], f32, tag="tp")
                    for j in range(4):
                        n = half * 4 + j
                        nc.tensor.transpose(
                            pt[:, j * P : (j + 1) * P],
                            h_tile[:, n * P : (n + 1) * P],
                            identity[:],
                        )
                    nc.scalar.copy(
                        out=ht[:, half * 512 : (half + 1) * 512], in_=pt[:]
                    )

                # out[tok, g] = sum_h ht[h, tok] * wt[h, g]
                op = mm_psum.tile([P, TOK], f32, tag="mm")
                for n in range(N_SUB):
                    nc.tensor.matmul(
                        op[:, n * P : (n + 1) * P],
                        lhsT=ht[:, n * P : (n + 1) * P],
                        rhs=wt[:],
                        start=True,
                        stop=True,
                    )

                o_tile = sbuf.tile([P, TOK], f32, tag="o_tile")
                nc.vector.tensor_add(out=o_tile[:], in0=op[:], in1=x_tile[:])
                nc.sync.dma_start(out=o_v[m], in_=o_tile[:])

    nc.compile()
    return nc


def _get_nc():
    if "nc" not in _CACHE:
        _CACHE["nc"] = _build_nc()
    return _CACHE["nc"]


def kernel(x_projected_t: np.ndarray, h_prev: np.ndarray, W_hh: np.ndarray) -> np.ndarray:
    from concourse.bass_utils import run_bass_kernel_spmd

    nc = _get_nc()

    x_projected_t = np.ascontiguousarray(x_projected_t, dtype=np.float32)
    h_prev = np.ascontiguousarray(h_prev, dtype=np.float32)
    W_hh = np.ascontiguousarray(W_hh, dtype=np.float32)

    in_maps = [
        {
            "x": x_projected_t[c * B_CORE : (c + 1) * B_CORE],
            "h": h_prev[c * B_CORE : (c + 1) * B_CORE],
            "w": W_hh,
        }
        for c in range(N_CORES)
    ]

    res = run_bass_kernel_spmd(nc, in_maps, core_ids=list(range(N_CORES)))
    return np.concatenate([res.results[c]["out"] for c in range(N_CORES)], axis=0)


# revision 7
# speedup vs baseline: 2.6300x; 2.6300x over previous
"""Trainium2 Bass kernel for nn_LinearRecurrentCell.

Computes h_next = h_prev @ W_hh.T + x_projected_t with
B = 1048576 tokens, H = 128, f32.

Strategy (data-parallel over 8 NeuronCores, per the sharding hint):
- Shard the batch dim of x/h across the 8 cores (131072 tokens each),
  replicate the 64 KB W_hh.
- Per core the kernel is memory-bound: 192 MiB of HBM traffic vs tiny
  compute. We stream 1024-token macro-tiles ([128 part x 1024 free] f32,
  512 KiB per DMA, 4 KiB contiguous per partition).
- The contraction over H needs H on SBUF partitions, so each 128x128
  sub-tile of h is PE-transposed (to PSUM), copied back to SBUF by the
  scalar engine, matmul'd against W^T (PE), then the vector engine adds
  x and the result streams back out.
"""

import numpy as np

B = 1048576
H = 128
P = 128
N_CORES = 8
B_CORE = B // N_CORES  # 131072
TOK = 1024             # tokens per macro-tile
N_SUB = TOK // P       # 8 sub-tiles of 128 tokens
N_MACRO = B_CORE // TOK  # 128 macro-tiles per core

_CACHE: dict = {}


def _build_nc(
    b_core: int = B_CORE,
    num_devices: int = N_CORES,
    n_repeat: int = 1,
    tok: int = TOK,
    sbuf_bufs: int = 3,
    store_eng: str = "sync",
    x_eng: str = "sync",
    copy_eng: str = "scalar",
):
    import concourse.mybir as mybir
    import concourse.tile as tile
    from concourse import bacc

    from concourse.masks import make_identity

    f32 = mybir.dt.float32
    n_sub = tok // P
    n_macro = b_core // tok

    nc = bacc.Bacc(
        "TRN2",
        target_bir_lowering=False,
        debug=False,
        enable_asserts=False,
        num_devices=num_devices,
    )

    x = nc.dram_tensor("x", [b_core, H], f32, kind="ExternalInput").ap()
    h = nc.dram_tensor("h", [b_core, H], f32, kind="ExternalInput").ap()
    w = nc.dram_tensor("w", [H, H], f32, kind="ExternalInput").ap()
    out = nc.dram_tensor("out", [b_core, H], f32, kind="ExternalOutput").ap()

    # Macro-tile views: token = m*tok + p*n_sub + n, so each partition's
    # slice of a macro-tile is n_sub*H*4 bytes contiguous in DRAM.
    x_v = x.rearrange("(m p n) h -> m p (n h)", p=P, n=n_sub)
    h_v = h.rearrange("(m p n) h -> m p (n h)", p=P, n=n_sub)
    o_v = out.rearrange("(m p n) h -> m p (n h)", p=P, n=n_sub)

    with tile.TileContext(nc) as tc:
        with (
            tc.tile_pool(name="consts", bufs=1) as consts,
            tc.tile_pool(name="sbuf", bufs=sbuf_bufs) as sbuf,
            tc.tile_pool(name="tp_psum", bufs=2, space="PSUM") as tp_psum,
            tc.tile_pool(name="mm_psum", bufs=2, space="PSUM") as mm_psum,
        ):
            store = {"sync": nc.sync, "scalar": nc.scalar, "gpsimd": nc.gpsimd}[
                store_eng
            ]
            x_load = {"sync": nc.sync, "scalar": nc.scalar, "gpsimd": nc.gpsimd}[
                x_eng
            ]
            copy = {"scalar": nc.scalar.copy, "vector": nc.vector.tensor_copy}[
                copy_eng
            ]

            identity = consts.tile([P, P], f32)
            make_identity(nc, identity[:])

            # Build W^T in SBUF once: wt[h, g] = W[g, h].
            w_nat = consts.tile([P, P], f32)
            nc.sync.dma_start(out=w_nat[:], in_=w[:, :])
            wt_psum = tp_psum.tile([P, 512], f32, tag="tp")
            nc.tensor.transpose(wt_psum[:, :P], w_nat[:], identity[:])
            wt = consts.tile([P, P], f32)
            nc.vector.tensor_copy(out=wt[:], in_=wt_psum[:, :P])

            for m in [mm for _ in range(n_repeat) for mm in range(n_macro)]:
                h_tile = sbuf.tile([P, tok], f32, tag="h_tile")
                x_tile = sbuf.tile([P, tok], f32, tag="x_tile")
                nc.sync.dma_start(out=h_tile[:], in_=h_v[m])
                x_load.dma_start(out=x_tile[:], in_=x_v[m])

                # Transpose the [128,128] sub-tiles: 4 at a time into one
                # PSUM bank, then one engine copy back to SBUF.
                ht = sbuf.tile([P, tok], f32, tag="ht")
                for half in range(tok // 512):
                    pt = tp_psum.tile([P, 512], f32, tag="tp")
                    for j in range(4):
                        n = half * 4 + j
                        nc.tensor.transpose(
                            pt[:, j * P : (j + 1) * P],
                            h_tile[:, n * P : (n + 1) * P],
                            identity[:],
                        )
                    copy(out=ht[:, half * 512 : (half + 1) * 512], in_=pt[:])

                # out[tok_p, g] = sum_h ht[h, tok_p] * wt[h, g]
                op = mm_psum.tile([P, tok], f32, tag="mm")
                for n in range(n_sub):
                    nc.tensor.matmul(
                        op[:, n * P : (n + 1) * P],
                        lhsT=ht[:, n * P : (n + 1) * P],
                        rhs=wt[:],
                        start=True,
                        stop=True,
                    )

                o_tile = sbuf.tile([P, tok], f32, tag="o_tile")
                nc.vector.tensor_add(out=o_tile[:], in0=op[:], in1=x_tile[:])
                store.dma_start(out=o_v[m], in_=o_tile[:])

    nc.compile()
    return nc


def _get_nc():
    if "nc" not in _CACHE:
        _CACHE["nc"] = _build_nc()
    return _CACHE["nc"]


def kernel(x_projected_t: np.ndarray, h_prev: np.ndarray, W_hh: np.ndarray) -> np.ndarray:
    from concourse.bass_utils import run_bass_kernel_spmd

    nc = _get_nc()

    x_projected_t = np.ascontiguousarray(x_projected_t, dtype=np.float32)
    h_prev = np.ascontiguousarray(h_prev, dtype=np.float32)
    W_hh = np.ascontiguousarray(W_hh, dtype=np.float32)

    in_maps = [
        {
            "x": x_projected_t[c * B_CORE : (c + 1) * B_CORE],
            "h": h_prev[c * B_CORE : (c + 1) * B_CORE],
            "w": W_hh,
        }
        for c in range(N_CORES)
    ]

    res = run_bass_kernel_spmd(nc, in_maps, core_ids=list(range(N_CORES)))
    return np.concatenate([res.results[c]["out"] for c in range(N_CORES)], axis=0)
